# revision 16
# baseline (speedup 1.0000x reference)
"""CGNN message-passing kernel for 8 trn2 NeuronCores.

Algorithm (per image (b,a), image = [S=768, T=14] grid):
  x = pw_vh(dw_hh(concat(h2,h1))) + pw_vp(dw_hp(pe)) + beta   (conv1 + pe branch)
  x = relu(x)
  y = pw_ov(dw_oh(x)) + beta2                                 (conv2)

Layout strategy: channel-major SBUF tiles [(chan,t) partitions, s free].
Depthwise 3x3 convs become 3 accumulating matmuls (one per s-shift ds) with
host-precomputed banded lhsT matrices that encode the t-direction taps
(T=14 blocks on the partition axis). conv2's depthwise+pointwise are fused
into a single banded lhsT per (u-chunk, ds). The pe branch and all biases
are folded into the conv1 pointwise contraction via a stacked rhs tile
[hd(84); pedw(28); ones(1)]. All matmuls run in bf16 (4x PE throughput);
PSUM accumulation stays fp32. Ingest/egress between DRAM pixel-major
layout and channel-major tiles uses PE transposes.

Sharding: data-parallel over batch B=16 -> 2 batches per core.
"""

import numpy as np
import ml_dtypes
from contextlib import ExitStack

import concourse.bass as bass
import concourse.bacc as bacc
import concourse.tile as tile
from concourse import mybir
from concourse.bass_utils import run_bass_kernel_spmd

F32 = mybir.dt.float32
BF16 = mybir.dt.bfloat16
NPBF = ml_dtypes.bfloat16
B, S, T, A = 16, 768, 14, 16
HK0, PEK0, U, K1 = 6, 2, 32, 2
NCORES = 8
BPC = B // NCORES          # batches per core
NST = S // 128             # 6 s-tiles of 128
SP = S + 2                 # s-padded width (zero col at 0 and S+1)
UCH = [9, 9, 9, 5]         # u-chunk sizes (32 = 9+9+9+5)
UOF = [0, 9, 18, 27]
# conv1 (dw + pw) s-chunks: chunk0 extends 2 cols so conv2 chunk0 only
# needs chunk0's relu output (incl. the s=384,385 halo cols).
CH1 = [(0, 386), (386, 382)]
# conv2 s-chunks
CH2 = [(0, 384), (384, 384)]


def _tband(w_t, n_t=T):
    """[n_t, n_t] band matrix M[t, t'] = w_t[t - t' + 1] (3-tap, SAME pad)."""
    m = np.zeros((n_t, n_t), np.float32)
    for t in range(n_t):
        for tp in range(n_t):
            dt = t - tp + 1
            if 0 <= dt <= 2:
                m[t, tp] = w_t[dt]
    return m


def build_consts(w_hh, b_hh, w_vh, b_vh, w_hp, b_hp, w_vp, b_vp,
                 w_oh, b_oh, w_ov, b_ov):
    """Host-side precompute of all lhsT matrices. Returns dict name->array."""
    w_hh = w_hh[:, :, 0, :]   # [3,3,6]
    w_hp = w_hp[:, :, 0, :]   # [3,3,2]
    w_oh = w_oh[:, :, 0, :]   # [3,3,32]

    # conv1 depthwise band: [3, 84, 84], rows/cols = g*14+t, g = concat chan
    B1 = np.zeros((3, 6 * T, 6 * T), np.float32)
    for ds in range(3):
        for g in range(6):
            B1[ds, g * T:(g + 1) * T, g * T:(g + 1) * T] = _tband(w_hh[ds, :, g])

    # conv1 pointwise with stacked pe rows + bias row: [125, 448]
    # rows 0:84 = hd rows (g,t'); 84:96 zero pad (32-aligned pe block);
    # 96:124 = pedw rows (c,t'); 124 = ones row (carries beta)
    ncol = sum(u * T for u in UCH)
    beta = (b_vh + w_vh.T @ b_hh + b_vp + w_vp.T @ b_hp).astype(np.float32)  # [32]
    W1 = np.zeros((96 + 2 * T + 1, ncol), np.float32)
    col = 0
    for uc in range(4):
        for ul in range(UCH[uc]):
            u = UOF[uc] + ul
            for g in range(6):
                W1[g * T:(g + 1) * T, col:col + T] = np.eye(T, dtype=np.float32) * w_vh[g, u]
            for c in range(2):
                W1[96 + c * T:96 + (c + 1) * T, col:col + T] = \
                    np.eye(T, dtype=np.float32) * w_vp[c, u]
            W1[96 + 2 * T, col:col + T] = beta[u]
            col += T

    # pe depthwise band with ones-passthrough row: [3, 29, 29]
    # rows/cols 0:28 = (c,t); row/col 28 = ones lane (identity at ds=1)
    Bpe = np.zeros((3, 2 * T + 1, 2 * T + 1), np.float32)
    for ds in range(3):
        for c in range(2):
            Bpe[ds, c * T:(c + 1) * T, c * T:(c + 1) * T] = _tband(w_hp[ds, :, c])
    Bpe[1, 2 * T, 2 * T] = 1.0

    # conv2 fused band (dw_oh folded with pw_ov): [126, 12*28]
    # col block (uc*3+ds): [uch*14, 28]; rows (u_local,t), cols (k,t')
    # Last uc block is 97 rows: 70 data + 26 zero pad + ones row at 96
    # (xs[3] row 96 is memset to 1.0) which carries beta2 into the psum.
    beta2 = (b_ov + w_ov.T @ b_oh).astype(np.float32)  # [2]
    B2 = np.zeros((126, 12 * (K1 * T)), np.float32)
    for uc in range(4):
        for ds in range(3):
            blk = np.zeros((UCH[uc] * T, K1 * T), np.float32)
            for ul in range(UCH[uc]):
                u = UOF[uc] + ul
                band = _tband(w_oh[ds, :, u])          # [T, T]
                for k in range(K1):
                    blk[ul * T:(ul + 1) * T, k * T:(k + 1) * T] = band * w_ov[u, k]
            c0 = (uc * 3 + ds) * (K1 * T)
            B2[:UCH[uc] * T, c0:c0 + K1 * T] = blk
    for k in range(K1):
        B2[96, (3 * 3 + 1) * (K1 * T) + k * T:(3 * 3 + 1) * (K1 * T) + (k + 1) * T] = beta2[k]

    return {
        "wI": np.eye(128, dtype=np.float32),
        "wIb": np.eye(128, dtype=np.float32).astype(NPBF),
        "wB1": B1.astype(NPBF), "wW1": W1.astype(NPBF),
        "wBpe": Bpe.astype(NPBF), "wB2": B2.astype(NPBF),
    }


def _trace_kernel(nc):
    h1 = nc.dram_tensor("h1", [BPC, S, T, A, 3], F32, kind="ExternalInput").ap()
    h2 = nc.dram_tensor("h2", [BPC, S, T, A, 3], F32, kind="ExternalInput").ap()
    pe = nc.dram_tensor("pe", [BPC, S, T, PEK0], F32, kind="ExternalInput").ap()
    wI = nc.dram_tensor("wI", [128, 128], F32, kind="ExternalInput").ap()
    wIb = nc.dram_tensor("wIb", [128, 128], BF16, kind="ExternalInput").ap()
    wB1 = nc.dram_tensor("wB1", [3, 84, 84], BF16, kind="ExternalInput").ap()
    wW1 = nc.dram_tensor("wW1", [125, 448], BF16, kind="ExternalInput").ap()
    wBpe = nc.dram_tensor("wBpe", [3, 29, 29], BF16, kind="ExternalInput").ap()
    wB2 = nc.dram_tensor("wB2", [126, 336], BF16, kind="ExternalInput").ap()
    y = nc.dram_tensor("y", [BPC, S, T, A, K1], F32, kind="ExternalOutput").ap()

    RELU = mybir.ActivationFunctionType.Relu

    with tile.TileContext(nc) as tc, ExitStack() as ctx:
        wp = ctx.enter_context(tc.tile_pool(name="w", bufs=1))
        hstage_p = ctx.enter_context(tc.tile_pool(name="hstage", bufs=2))
        pestage_p = ctx.enter_context(tc.tile_pool(name="pestage", bufs=2))
        hcp_p = ctx.enter_context(tc.tile_pool(name="hcp", bufs=2))
        pcp_p = ctx.enter_context(tc.tile_pool(name="pcp", bufs=2))
        ht_p = ctx.enter_context(tc.tile_pool(name="ht", bufs=2))
        stk_p = ctx.enter_context(tc.tile_pool(name="stk", bufs=2))
        xs_p = ctx.enter_context(tc.tile_pool(name="xs", bufs=2))
        yt_p = ctx.enter_context(tc.tile_pool(name="ysb", bufs=2))
        stout_p = ctx.enter_context(tc.tile_pool(name="stout", bufs=2))
        psA = ctx.enter_context(tc.tile_pool(name="psA", bufs=2, space="PSUM"))
        psB = ctx.enter_context(tc.tile_pool(name="psB", bufs=4, space="PSUM"))
        psC = ctx.enter_context(tc.tile_pool(name="psC", bufs=2, space="PSUM"))

        ident = wp.tile([128, 128], F32)
        nc.sync.dma_start(ident[:], wI)
        identb = wp.tile([128, 128], BF16)
        nc.sync.dma_start(identb[:], wIb)
        b1 = wp.tile([84, 3, 84], BF16)
        nc.sync.dma_start(b1[:], wB1.rearrange("d k m -> k d m"))
        w1 = wp.tile([125, 448], BF16)
        nc.sync.dma_start(w1[:], wW1)
        bpe = wp.tile([29, 3, 29], BF16)
        nc.sync.dma_start(bpe[:], wBpe.rearrange("d k m -> k d m"))
        b2t = wp.tile([126, 336], BF16)
        nc.sync.dma_start(b2t[:], wB2)
        for b in range(BPC):
            hs = hstage_p.tile([128, 2, NST, T, A, 3], F32, tag="hs")
            for n in range(NST):
                nc.sync.dma_start(hs[:, 0, n], h2[b, n * 128:(n + 1) * 128])
                nc.sync.dma_start(hs[:, 1, n], h1[b, n * 128:(n + 1) * 128])
            ps = pestage_p.tile([128, NST, T, PEK0], F32, tag="ps")
            nc.sync.dma_start(ps[:], pe[b].rearrange("(n p) t c -> p n t c", p=128))

            # ---- per-antenna images (software-pipelined ingest) ----
            def ingest(aa):
                """Stage image aa: strided cast copy, PE transposes to
                channel-major, copy out of PSUM with s-halo. Returns hT.
                For the first antenna the copies+transposes go per s-tile so
                the PE can start as soon as the first DMA chunks land."""
                hcp = hcp_p.tile([128, NST, 2, 3, T], BF16, tag="hcp",
                                 name=f"hcp{aa}")
                tp = psA.tile([84, S], BF16, tag="s", name=f"tp{aa}")
                if aa == 0:
                    for j in range(NST):
                        nc.gpsimd.tensor_copy(
                            hcp[:, j, 0], hs[:, 0, j, :, aa, :].rearrange("p t c -> p c t"))
                        nc.vector.tensor_copy(
                            hcp[:, j, 1], hs[:, 1, j, :, aa, :].rearrange("p t c -> p c t"))
                        nc.tensor.transpose(tp[:, j * 128:(j + 1) * 128],
                                            hcp[:, j], identb[:])
                else:
                    nc.gpsimd.tensor_copy(
                        hcp[:, :, 0], hs[:, 0, :, :, aa, :].rearrange("p n t c -> p n c t"))
                    nc.gpsimd.tensor_copy(
                        hcp[:, :, 1], hs[:, 1, :, :, aa, :].rearrange("p n t c -> p n c t"))
                    for j in range(NST):
                        nc.tensor.transpose(tp[:, j * 128:(j + 1) * 128],
                                            hcp[:, j], identb[:])
                hT = ht_p.tile([84, SP], BF16, tag="hT", name=f"hT{aa}")
                if aa < 2:
                    nc.gpsimd.memset(hT[:, 0:1], 0.0)
                    nc.gpsimd.memset(hT[:, SP - 1:SP], 0.0)
                nc.vector.tensor_copy(hT[:, 1:385], tp[:, 0:384])
                nc.scalar.copy(hT[:, 385:769], tp[:, 384:768])
                return hT

            hT_next = ingest(0)
            # ---- pe branch (per b, shared by all 16 antennas) ----
            # pcp carries a ones lane (col 28) that rides through the
            # transpose + dw band so the stacked pw tile gets its bias row.
            pcp = pcp_p.tile([128, NST, PEK0 * T + 1], BF16, tag="pcp")
            nc.vector.tensor_copy(
                pcp[:, :, 0:28].rearrange("p n (c t) -> p n c t", c=PEK0),
                ps[:].rearrange("p n t c -> p n c t"))
            nc.vector.memset(pcp[:, :, 28:29], 1.0)
            tpe = psA.tile([29, S], BF16, tag="s")
            for j in range(NST):
                nc.tensor.transpose(tpe[0:29, j * 128:(j + 1) * 128],
                                    pcp[:, j], identb[:])
            peT = ht_p.tile([29, SP], BF16, tag="peT")
            nc.vector.memset(peT[:, 0:1], 0.0)
            nc.gpsimd.memset(peT[:, SP - 1:SP], 0.0)
            nc.vector.tensor_copy(peT[0:29, 1:385], tpe[0:29, 0:384])
            nc.scalar.copy(peT[0:29, 385:769], tpe[0:29, 384:768])

            stk0 = stk_p.tile([125, S], BF16, tag="stk0")
            stk1 = stk_p.tile([125, S], BF16, tag="stk1")
            nc.vector.memset(stk0[64:96, :], 0.0)
            nc.gpsimd.memset(stk1[64:96, :], 0.0)
            for s0, sn in CH1:
                dqp = psA.tile([29, sn], F32, tag="s")
                for ds in range(3):
                    nc.tensor.matmul(dqp[0:29, :sn], bpe[:, ds, :],
                                     peT[:, ds + s0: ds + s0 + sn],
                                     start=(ds == 0), stop=(ds == 2))
                nc.scalar.copy(stk0[96:125, s0:s0 + sn], dqp[0:29, :sn])
                nc.vector.tensor_copy(stk1[96:125, s0:s0 + sn], dqp[0:29, :sn])

            stout = stout_p.tile([128, NST, T, A, K1], F32, tag="so")

            for a in range(A):
                stk = stk0 if a % 2 == 0 else stk1
                hT = hT_next

                # dw conv1 -> stack rows 0:84
                for ci, (s0, sn) in enumerate(CH1):
                    dq = psA.tile([84, sn], F32, tag="s")
                    for ds in range(3):
                        nc.tensor.matmul(dq[:, :sn], b1[:, ds, :],
                                         hT[:, ds + s0: ds + s0 + sn],
                                         start=(ds == 0), stop=(ds == 2))
                    if ci == 0:
                        nc.scalar.copy(stk[0:84, s0:s0 + sn], dq[:, :sn])
                    else:
                        nc.vector.tensor_copy(stk[0:84, s0:s0 + sn], dq[:, :sn])

                # pw conv1 (+pe+bias via stacked rhs), relu -> xs
                # xs[3] gets 27 extra rows: zeros at 70:96, ones at 96 —
                # the conv2 lhsT ones-row picks beta2 up from there.
                xts = []
                for uc in range(4):
                    m = 97 if uc == 3 else UCH[uc] * T
                    xt = xs_p.tile([m, SP], BF16, tag=f"x{uc}")
                    xts.append(xt)
                    if a < 2:
                        eng = nc.vector if uc % 2 == 0 else nc.gpsimd
                        eng.memset(xt[:, 0:1], 0.0)
                        eng.memset(xt[:, SP - 1:SP], 0.0)
                        if uc == 3:
                            nc.gpsimd.memset(xt[64:96, :], 0.0)
                            nc.gpsimd.memset(xt[96:97, :], 1.0)
                for s0, sn in CH1:
                    for uc in range(4):
                        m = UCH[uc] * T
                        c0 = UOF[uc] * T
                        px = psB.tile([126, sn], F32, tag="x")
                        nc.tensor.matmul(px[0:m, :sn], w1[:, c0:c0 + m],
                                         stk[:, s0:s0 + sn], start=True, stop=True)
                        dst = xts[uc][0:m, 1 + s0:1 + s0 + sn]
                        if uc == 0 or uc == 2:
                            nc.scalar.activation(dst, px[0:m, :sn], RELU)
                        else:
                            nc.vector.tensor_scalar_max(dst, px[0:m, :sn], 0.0)

                # ingest the next image while this one's relus drain
                if a + 1 < A:
                    hT_next = ingest(a + 1)

                # conv2 (fused dw+pw banded lhsT, beta2 via xs[3] ones-row)
                # Image pairs share one yt tile (rows 0:28 even / 32:60 odd)
                # so the egress transposes run once per pair.
                if a % 2 == 0:
                    yt_pair = yt_p.tile([60, S], F32, tag="yt")
                r0 = 0 if a % 2 == 0 else 32
                for ci, (s0, sn) in enumerate(CH2):
                    yq = psC.tile([28, sn], F32, tag="y")
                    kk = 0
                    for uc in range(4):
                        m = 97 if uc == 3 else UCH[uc] * T
                        for ds in range(3):
                            c0 = (uc * 3 + ds) * 28
                            nc.tensor.matmul(yq[:, :sn], b2t[0:m, c0:c0 + 28],
                                             xts[uc][0:m, ds + s0: ds + s0 + sn],
                                             start=(kk == 0), stop=(kk == 11))
                            kk += 1
                    if ci == 0:
                        nc.scalar.copy(yt_pair[r0:r0 + 28, s0:s0 + sn], yq[:, :sn])
                    else:
                        nc.vector.tensor_copy(yt_pair[r0:r0 + 28, s0:s0 + sn], yq[:, :sn])

                # egress (once per pair): transpose to pixel-major, scatter
                if a % 2 == 1:
                    tp2 = psC.tile([128, NST, 60], F32, tag="y")
                    for st in range(NST):
                        nc.tensor.transpose(tp2[:, st],
                                            yt_pair[0:60, st * 128:(st + 1) * 128],
                                            ident[0:60, 0:60])
                    for aa, rr in ((a - 1, 0), (a, 32)):
                        for g in range(2):
                            src = tp2[:, g * 3:(g + 1) * 3, rr:rr + 28].rearrange(
                                "p n (k t) -> p n t k", k=K1)
                            nc.scalar.copy(stout[:, g * 3:g * 3 + 3, :, aa, :], src)

            nc.sync.dma_start(y[b].rearrange("(n p) t a k -> p n t a k", p=128),
                              stout[:])
    nc.compile()
    return nc


_CACHED_NC = None


def get_nc():
    global _CACHED_NC
    if _CACHED_NC is None:
        _CACHED_NC = _trace_kernel(
            bacc.Bacc("TRN2", target_bir_lowering=False, debug=False))
    return _CACHED_NC


def make_in_maps(inputs):
    consts = build_consts(
        inputs["w_hh"], inputs["b_hh"], inputs["w_vh"], inputs["b_vh"],
        inputs["w_hp"], inputs["b_hp"], inputs["w_vp"], inputs["b_vp"],
        inputs["w_oh"], inputs["b_oh"], inputs["w_ov"], inputs["b_ov"])
    in_maps = []
    for i in range(NCORES):
        sl = slice(i * BPC, (i + 1) * BPC)
        m = {
            "h1": np.ascontiguousarray(inputs["h1"][sl], np.float32),
            "h2": np.ascontiguousarray(inputs["h2"][sl], np.float32),
            "pe": np.ascontiguousarray(inputs["pe"][sl], np.float32),
        }
        m.update(consts)
        in_maps.append(m)
    return in_maps


def kernel(**inputs):
    nc = get_nc()
    in_maps = make_in_maps(inputs)
    res = run_bass_kernel_spmd(nc, in_maps, list(range(NCORES)))
    return np.concatenate([r["y"] for r in res.results], axis=0)


# revision 17
# speedup vs baseline: 1.1440x; 1.1440x over previous
"""CGNN message-passing kernel for 8 trn2 NeuronCores.

Algorithm (per image (b,a), image = [S=768, T=14] grid):
  x = pw_vh(dw_hh(concat(h2,h1))) + pw_vp(dw_hp(pe)) + beta   (conv1 + pe branch)
  x = relu(x)
  y = pw_ov(dw_oh(x)) + beta2                                 (conv2)

Layout strategy: channel-major SBUF tiles [(chan,t) partitions, s free].
Depthwise 3x3 convs become 3 accumulating matmuls (one per s-shift ds) with
host-precomputed banded lhsT matrices that encode the t-direction taps
(T=14 blocks on the partition axis). conv2's depthwise+pointwise are fused
into a single banded lhsT per (u-chunk, ds). The pe branch and all biases
are folded into the conv1 pointwise contraction via a stacked rhs tile
[hd(84); pedw(28); ones(1)]. All matmuls run in bf16 (4x PE throughput);
PSUM accumulation stays fp32. Ingest/egress between DRAM pixel-major
layout and channel-major tiles uses PE transposes.

Sharding: data-parallel over batch B=16 -> 2 batches per core.
"""

import numpy as np
import ml_dtypes
from contextlib import ExitStack

import concourse.bass as bass
import concourse.bacc as bacc
import concourse.tile as tile
from concourse import mybir
from concourse.bass_utils import run_bass_kernel_spmd

F32 = mybir.dt.float32
BF16 = mybir.dt.bfloat16
NPBF = ml_dtypes.bfloat16
B, S, T, A = 16, 768, 14, 16
HK0, PEK0, U, K1 = 6, 2, 32, 2
NCORES = 8
BPC = B // NCORES          # batches per core
NST = S // 128             # 6 s-tiles of 128
SP = S + 2                 # s-padded width (zero col at 0 and S+1)
UCH = [9, 9, 9, 5]         # u-chunk sizes (32 = 9+9+9+5)
UOF = [0, 9, 18, 27]
# conv1 (dw + pw) s-chunks: chunk0 extends 2 cols so conv2 chunk0 only
# needs chunk0's relu output (incl. the s=384,385 halo cols).
CH1 = [(0, 386), (386, 382)]
# conv2 s-chunks
CH2 = [(0, 384), (384, 384)]


def _tband(w_t, n_t=T):
    """[n_t, n_t] band matrix M[t, t'] = w_t[t - t' + 1] (3-tap, SAME pad)."""
    m = np.zeros((n_t, n_t), np.float32)
    for t in range(n_t):
        for tp in range(n_t):
            dt = t - tp + 1
            if 0 <= dt <= 2:
                m[t, tp] = w_t[dt]
    return m


def build_consts(w_hh, b_hh, w_vh, b_vh, w_hp, b_hp, w_vp, b_vp,
                 w_oh, b_oh, w_ov, b_ov):
    """Host-side precompute of all lhsT matrices. Returns dict name->array."""
    w_hh = w_hh[:, :, 0, :]   # [3,3,6]
    w_hp = w_hp[:, :, 0, :]   # [3,3,2]
    w_oh = w_oh[:, :, 0, :]   # [3,3,32]

    # conv1 depthwise band: [3, 84, 84], rows/cols = g*14+t, g = concat chan
    B1 = np.zeros((3, 6 * T, 6 * T), np.float32)
    for ds in range(3):
        for g in range(6):
            B1[ds, g * T:(g + 1) * T, g * T:(g + 1) * T] = _tband(w_hh[ds, :, g])

    # conv1 pointwise with stacked pe rows + bias row: [125, 448]
    # rows 0:84 = hd rows (g,t'); 84:96 zero pad (32-aligned pe block);
    # 96:124 = pedw rows (c,t'); 124 = ones row (carries beta)
    ncol = sum(u * T for u in UCH)
    beta = (b_vh + w_vh.T @ b_hh + b_vp + w_vp.T @ b_hp).astype(np.float32)  # [32]
    W1 = np.zeros((96 + 2 * T + 1, ncol), np.float32)
    col = 0
    for uc in range(4):
        for ul in range(UCH[uc]):
            u = UOF[uc] + ul
            for g in range(6):
                W1[g * T:(g + 1) * T, col:col + T] = np.eye(T, dtype=np.float32) * w_vh[g, u]
            for c in range(2):
                W1[96 + c * T:96 + (c + 1) * T, col:col + T] = \
                    np.eye(T, dtype=np.float32) * w_vp[c, u]
            W1[96 + 2 * T, col:col + T] = beta[u]
            col += T

    # pe depthwise band with ones-passthrough row: [3, 29, 29]
    # rows/cols 0:28 = (c,t); row/col 28 = ones lane (identity at ds=1)
    Bpe = np.zeros((3, 2 * T + 1, 2 * T + 1), np.float32)
    for ds in range(3):
        for c in range(2):
            Bpe[ds, c * T:(c + 1) * T, c * T:(c + 1) * T] = _tband(w_hp[ds, :, c])
    Bpe[1, 2 * T, 2 * T] = 1.0

    # conv2 fused band (dw_oh folded with pw_ov): [126, 12*28]
    # col block (uc*3+ds): [uch*14, 28]; rows (u_local,t), cols (k,t')
    # Last uc block is 97 rows: 70 data + 26 zero pad + ones row at 96
    # (xs[3] row 96 is memset to 1.0) which carries beta2 into the psum.
    beta2 = (b_ov + w_ov.T @ b_oh).astype(np.float32)  # [2]
    B2 = np.zeros((126, 12 * (K1 * T)), np.float32)
    for uc in range(4):
        for ds in range(3):
            blk = np.zeros((UCH[uc] * T, K1 * T), np.float32)
            for ul in range(UCH[uc]):
                u = UOF[uc] + ul
                band = _tband(w_oh[ds, :, u])          # [T, T]
                for k in range(K1):
                    blk[ul * T:(ul + 1) * T, k * T:(k + 1) * T] = band * w_ov[u, k]
            c0 = (uc * 3 + ds) * (K1 * T)
            B2[:UCH[uc] * T, c0:c0 + K1 * T] = blk
    for k in range(K1):
        B2[96, (3 * 3 + 1) * (K1 * T) + k * T:(3 * 3 + 1) * (K1 * T) + (k + 1) * T] = beta2[k]

    return {
        "wI": np.eye(128, dtype=np.float32),
        "wIb": np.eye(128, dtype=np.float32).astype(NPBF),
        "wB1": B1.astype(NPBF), "wW1": W1.astype(NPBF),
        "wBpe": Bpe.astype(NPBF), "wB2": B2.astype(NPBF),
    }


def _trace_kernel(nc):
    h1 = nc.dram_tensor("h1", [BPC, S, T, A, 3], F32, kind="ExternalInput").ap()
    h2 = nc.dram_tensor("h2", [BPC, S, T, A, 3], F32, kind="ExternalInput").ap()
    pe = nc.dram_tensor("pe", [BPC, S, T, PEK0], F32, kind="ExternalInput").ap()
    wI = nc.dram_tensor("wI", [128, 128], F32, kind="ExternalInput").ap()
    wIb = nc.dram_tensor("wIb", [128, 128], BF16, kind="ExternalInput").ap()
    wB1 = nc.dram_tensor("wB1", [3, 84, 84], BF16, kind="ExternalInput").ap()
    wW1 = nc.dram_tensor("wW1", [125, 448], BF16, kind="ExternalInput").ap()
    wBpe = nc.dram_tensor("wBpe", [3, 29, 29], BF16, kind="ExternalInput").ap()
    wB2 = nc.dram_tensor("wB2", [126, 336], BF16, kind="ExternalInput").ap()
    y = nc.dram_tensor("y", [BPC, S, T, A, K1], F32, kind="ExternalOutput").ap()

    RELU = mybir.ActivationFunctionType.Relu

    with tile.TileContext(nc) as tc, ExitStack() as ctx:
        wp = ctx.enter_context(tc.tile_pool(name="w", bufs=1))
        hstage_p = ctx.enter_context(tc.tile_pool(name="hstage", bufs=2))
        pestage_p = ctx.enter_context(tc.tile_pool(name="pestage", bufs=2))
        hcp_p = ctx.enter_context(tc.tile_pool(name="hcp", bufs=2))
        pcp_p = ctx.enter_context(tc.tile_pool(name="pcp", bufs=2))
        ht_p = ctx.enter_context(tc.tile_pool(name="ht", bufs=2))
        stk_p = ctx.enter_context(tc.tile_pool(name="stk", bufs=2))
        xs_p = ctx.enter_context(tc.tile_pool(name="xs", bufs=2))
        yt_p = ctx.enter_context(tc.tile_pool(name="ysb", bufs=2))
        stout_p = ctx.enter_context(tc.tile_pool(name="stout", bufs=2))
        psA = ctx.enter_context(tc.tile_pool(name="psA", bufs=2, space="PSUM"))
        psB = ctx.enter_context(tc.tile_pool(name="psB", bufs=4, space="PSUM"))
        psC = ctx.enter_context(tc.tile_pool(name="psC", bufs=2, space="PSUM"))

        ident = wp.tile([128, 128], F32)
        nc.sync.dma_start(ident[:], wI)
        identb = wp.tile([128, 128], BF16)
        nc.sync.dma_start(identb[:], wIb)
        b1 = wp.tile([84, 3, 84], BF16)
        nc.sync.dma_start(b1[:], wB1.rearrange("d k m -> k d m"))
        w1 = wp.tile([125, 448], BF16)
        nc.sync.dma_start(w1[:], wW1)
        bpe = wp.tile([29, 3, 29], BF16)
        nc.sync.dma_start(bpe[:], wBpe.rearrange("d k m -> k d m"))
        b2t = wp.tile([126, 336], BF16)
        nc.sync.dma_start(b2t[:], wB2)
        for b in range(BPC):
            hs = hstage_p.tile([128, 2, NST, T, A, 3], F32, tag="hs")
            for n in range(NST):
                nc.sync.dma_start(hs[:, 0, n], h2[b, n * 128:(n + 1) * 128])
                nc.sync.dma_start(hs[:, 1, n], h1[b, n * 128:(n + 1) * 128])
            ps = pestage_p.tile([128, NST, T, PEK0], F32, tag="ps")
            nc.sync.dma_start(ps[:], pe[b].rearrange("(n p) t c -> p n t c", p=128))

            # ---- per-antenna images (software-pipelined ingest) ----
            def ingest(aa):
                """Stage image aa: strided cast copy, PE transposes to
                channel-major, copy out of PSUM with s-halo. Returns hT.
                For the first antenna the copies+transposes go per s-tile so
                the PE can start as soon as the first DMA chunks land."""
                hcp = hcp_p.tile([128, NST, 2, 3, T], BF16, tag="hcp",
                                 name=f"hcp{aa}")
                tp = psA.tile([84, S], BF16, tag="s", name=f"tp{aa}")
                if aa == 0:
                    for j in range(NST):
                        nc.gpsimd.tensor_copy(
                            hcp[:, j, 0], hs[:, 0, j, :, aa, :].rearrange("p t c -> p c t"))
                        nc.vector.tensor_copy(
                            hcp[:, j, 1], hs[:, 1, j, :, aa, :].rearrange("p t c -> p c t"))
                        nc.tensor.transpose(tp[:, j * 128:(j + 1) * 128],
                                            hcp[:, j], identb[:])
                else:
                    nc.gpsimd.tensor_copy(
                        hcp[:, :, 0], hs[:, 0, :, :, aa, :].rearrange("p n t c -> p n c t"))
                    nc.gpsimd.tensor_copy(
                        hcp[:, :, 1], hs[:, 1, :, :, aa, :].rearrange("p n t c -> p n c t"))
                    for j in range(NST):
                        nc.tensor.transpose(tp[:, j * 128:(j + 1) * 128],
                                            hcp[:, j], identb[:])
                hT = ht_p.tile([84, SP], BF16, tag="hT", name=f"hT{aa}")
                if aa < 2:
                    nc.gpsimd.memset(hT[:, 0:1], 0.0)
                    nc.gpsimd.memset(hT[:, SP - 1:SP], 0.0)
                nc.vector.tensor_copy(hT[:, 1:385], tp[:, 0:384])
                nc.scalar.copy(hT[:, 385:769], tp[:, 384:768])
                return hT

            hT_next = ingest(0)
            # ---- pe branch (per b, shared by all 16 antennas) ----
            # pcp carries a ones lane (col 28) that rides through the
            # transpose + dw band so the stacked pw tile gets its bias row.
            pcp = pcp_p.tile([128, NST, PEK0 * T + 1], BF16, tag="pcp")
            nc.vector.tensor_copy(
                pcp[:, :, 0:28].rearrange("p n (c t) -> p n c t", c=PEK0),
                ps[:].rearrange("p n t c -> p n c t"))
            nc.vector.memset(pcp[:, :, 28:29], 1.0)
            tpe = psA.tile([29, S], BF16, tag="s")
            for j in range(NST):
                nc.tensor.transpose(tpe[0:29, j * 128:(j + 1) * 128],
                                    pcp[:, j], identb[:])
            peT = ht_p.tile([29, SP], BF16, tag="peT")
            nc.vector.memset(peT[:, 0:1], 0.0)
            nc.gpsimd.memset(peT[:, SP - 1:SP], 0.0)
            nc.vector.tensor_copy(peT[0:29, 1:385], tpe[0:29, 0:384])
            nc.scalar.copy(peT[0:29, 385:769], tpe[0:29, 384:768])

            stk0 = stk_p.tile([125, S], BF16, tag="stk0")
            stk1 = stk_p.tile([125, S], BF16, tag="stk1")
            nc.vector.memset(stk0[64:96, :], 0.0)
            nc.gpsimd.memset(stk1[64:96, :], 0.0)
            for s0, sn in CH1:
                dqp = psA.tile([29, sn], F32, tag="s")
                for ds in range(3):
                    nc.tensor.matmul(dqp[0:29, :sn], bpe[:, ds, :],
                                     peT[:, ds + s0: ds + s0 + sn],
                                     start=(ds == 0), stop=(ds == 2))
                nc.scalar.copy(stk0[96:125, s0:s0 + sn], dqp[0:29, :sn])
                nc.vector.tensor_copy(stk1[96:125, s0:s0 + sn], dqp[0:29, :sn])

            stout = stout_p.tile([128, NST, T, A, K1], F32, tag="so")

            for a in range(A):
                stk = stk0 if a % 2 == 0 else stk1
                hT = hT_next

                # dw conv1 -> stack rows 0:84
                for ci, (s0, sn) in enumerate(CH1):
                    dq = psA.tile([84, sn], F32, tag="s")
                    for ds in range(3):
                        nc.tensor.matmul(dq[:, :sn], b1[:, ds, :],
                                         hT[:, ds + s0: ds + s0 + sn],
                                         start=(ds == 0), stop=(ds == 2))
                    if ci == 0:
                        nc.scalar.copy(stk[0:84, s0:s0 + sn], dq[:, :sn])
                    else:
                        nc.vector.tensor_copy(stk[0:84, s0:s0 + sn], dq[:, :sn])

                # pw conv1 (+pe+bias via stacked rhs), relu -> xs
                # xs[3] gets 27 extra rows: zeros at 70:96, ones at 96 —
                # the conv2 lhsT ones-row picks beta2 up from there.
                xts = []
                for uc in range(4):
                    m = 97 if uc == 3 else UCH[uc] * T
                    xt = xs_p.tile([m, SP], BF16, tag=f"x{uc}")
                    xts.append(xt)
                    if a < 2:
                        eng = nc.vector if uc % 2 == 0 else nc.gpsimd
                        eng.memset(xt[:, 0:1], 0.0)
                        eng.memset(xt[:, SP - 1:SP], 0.0)
                        if uc == 3:
                            nc.gpsimd.memset(xt[64:96, :], 0.0)
                            nc.gpsimd.memset(xt[96:97, :], 1.0)
                for s0, sn in CH1:
                    for uc in range(4):
                        m = UCH[uc] * T
                        c0 = UOF[uc] * T
                        px = psB.tile([126, sn], F32, tag="x")
                        nc.tensor.matmul(px[0:m, :sn], w1[:, c0:c0 + m],
                                         stk[:, s0:s0 + sn], start=True, stop=True)
                        dst = xts[uc][0:m, 1 + s0:1 + s0 + sn]
                        if uc == 0 or uc == 2:
                            nc.scalar.activation(dst, px[0:m, :sn], RELU)
                        else:
                            nc.vector.tensor_scalar_max(dst, px[0:m, :sn], 0.0)

                # ingest the next image while this one's relus drain
                if a + 1 < A:
                    hT_next = ingest(a + 1)

                # conv2 (fused dw+pw banded lhsT, beta2 via xs[3] ones-row)
                yt = yt_p.tile([28, S], F32, tag="yt")
                for ci, (s0, sn) in enumerate(CH2):
                    yq = psC.tile([28, sn], F32, tag="y")
                    kk = 0
                    for uc in range(4):
                        m = 97 if uc == 3 else UCH[uc] * T
                        for ds in range(3):
                            c0 = (uc * 3 + ds) * 28
                            nc.tensor.matmul(yq[:, :sn], b2t[0:m, c0:c0 + 28],
                                             xts[uc][0:m, ds + s0: ds + s0 + sn],
                                             start=(kk == 0), stop=(kk == 11))
                            kk += 1
                    if ci == 0:
                        nc.scalar.copy(yt[:, s0:s0 + sn], yq[:, :sn])
                    else:
                        nc.vector.tensor_copy(yt[:, s0:s0 + sn], yq[:, :sn])

                # egress: transpose back to pixel-major, scatter into stout
                tp2 = psC.tile([128, 168], F32, tag="y")
                for g in range(2):
                    for j in range(3):
                        st = g * 3 + j
                        nc.tensor.transpose(tp2[:, st * 28:(st + 1) * 28],
                                            yt[:, st * 128:(st + 1) * 128],
                                            ident[0:28, 0:28])
                    src = tp2[:, g * 84:(g + 1) * 84].rearrange(
                        "p (n k t) -> p n t k", n=3, k=K1)
                    nc.scalar.copy(stout[:, g * 3:g * 3 + 3, :, a, :], src)

            nc.sync.dma_start(y[b].rearrange("(n p) t a k -> p n t a k", p=128),
                              stout[:])
    nc.compile()
    return nc


_CACHED_NC = None


def get_nc():
    global _CACHED_NC
    if _CACHED_NC is None:
        _CACHED_NC = _trace_kernel(
            bacc.Bacc("TRN2", target_bir_lowering=False, debug=False))
    return _CACHED_NC


def make_in_maps(inputs):
    consts = build_consts(
        inputs["w_hh"], inputs["b_hh"], inputs["w_vh"], inputs["b_vh"],
        inputs["w_hp"], inputs["b_hp"], inputs["w_vp"], inputs["b_vp"],
        inputs["w_oh"], inputs["b_oh"], inputs["w_ov"], inputs["b_ov"])
    in_maps = []
    for i in range(NCORES):
        sl = slice(i * BPC, (i + 1) * BPC)
        m = {
            "h1": np.ascontiguousarray(inputs["h1"][sl], np.float32),
            "h2": np.ascontiguousarray(inputs["h2"][sl], np.float32),
            "pe": np.ascontiguousarray(inputs["pe"][sl], np.float32),
        }
        m.update(consts)
        in_maps.append(m)
    return in_maps


def kernel(**inputs):
    nc = get_nc()
    in_maps = make_in_maps(inputs)
    res = run_bass_kernel_spmd(nc, in_maps, list(range(NCORES)))
    return np.concatenate([r["y"] for r in res.results], axis=0)


# revision 18
# speedup vs baseline: 1.1459x; 1.0017x over previous
"""CGNN message-passing kernel for 8 trn2 NeuronCores.

Algorithm (per image (b,a), image = [S=768, T=14] grid):
  x = pw_vh(dw_hh(concat(h2,h1))) + pw_vp(dw_hp(pe)) + beta   (conv1 + pe branch)
  x = relu(x)
  y = pw_ov(dw_oh(x)) + beta2                                 (conv2)

Layout strategy: channel-major SBUF tiles [(chan,t) partitions, s free].
Depthwise 3x3 convs become 3 accumulating matmuls (one per s-shift ds) with
host-precomputed banded lhsT matrices that encode the t-direction taps
(T=14 blocks on the partition axis). conv2's depthwise+pointwise are fused
into a single banded lhsT per (u-chunk, ds). The pe branch and all biases
are folded into the conv1 pointwise contraction via a stacked rhs tile
[hd(84); pedw(28); ones(1)]. All matmuls run in bf16 (4x PE throughput);
PSUM accumulation stays fp32. Ingest/egress between DRAM pixel-major
layout and channel-major tiles uses PE transposes.

Sharding: data-parallel over batch B=16 -> 2 batches per core.
"""

import numpy as np
import ml_dtypes
from contextlib import ExitStack

import concourse.bass as bass
import concourse.bacc as bacc
import concourse.tile as tile
from concourse import mybir
from concourse.bass_utils import run_bass_kernel_spmd

F32 = mybir.dt.float32
BF16 = mybir.dt.bfloat16
NPBF = ml_dtypes.bfloat16
B, S, T, A = 16, 768, 14, 16
HK0, PEK0, U, K1 = 6, 2, 32, 2
NCORES = 8
BPC = B // NCORES          # batches per core
NST = S // 128             # 6 s-tiles of 128
SP = S + 2                 # s-padded width (zero col at 0 and S+1)
UCH = [9, 9, 9, 5]         # u-chunk sizes (32 = 9+9+9+5)
UOF = [0, 9, 18, 27]
# conv1 (dw + pw) s-chunks: chunk0 extends 2 cols so conv2 chunk0 only
# needs chunk0's relu output (incl. the s=384,385 halo cols).
CH1 = [(0, 386), (386, 382)]
# conv2 s-chunks
CH2 = [(0, 384), (384, 384)]


def _tband(w_t, n_t=T):
    """[n_t, n_t] band matrix M[t, t'] = w_t[t - t' + 1] (3-tap, SAME pad)."""
    m = np.zeros((n_t, n_t), np.float32)
    for t in range(n_t):
        for tp in range(n_t):
            dt = t - tp + 1
            if 0 <= dt <= 2:
                m[t, tp] = w_t[dt]
    return m


def build_consts(w_hh, b_hh, w_vh, b_vh, w_hp, b_hp, w_vp, b_vp,
                 w_oh, b_oh, w_ov, b_ov):
    """Host-side precompute of all lhsT matrices. Returns dict name->array."""
    w_hh = w_hh[:, :, 0, :]   # [3,3,6]
    w_hp = w_hp[:, :, 0, :]   # [3,3,2]
    w_oh = w_oh[:, :, 0, :]   # [3,3,32]

    # conv1 depthwise band: [3, 84, 84], rows/cols = g*14+t, g = concat chan
    B1 = np.zeros((3, 6 * T, 6 * T), np.float32)
    for ds in range(3):
        for g in range(6):
            B1[ds, g * T:(g + 1) * T, g * T:(g + 1) * T] = _tband(w_hh[ds, :, g])

    # conv1 pointwise with stacked pe rows + bias row: [125, 448]
    # rows 0:84 = hd rows (g,t'); 84:96 zero pad (32-aligned pe block);
    # 96:124 = pedw rows (c,t'); 124 = ones row (carries beta)
    ncol = sum(u * T for u in UCH)
    beta = (b_vh + w_vh.T @ b_hh + b_vp + w_vp.T @ b_hp).astype(np.float32)  # [32]
    W1 = np.zeros((96 + 2 * T + 1, ncol), np.float32)
    col = 0
    for uc in range(4):
        for ul in range(UCH[uc]):
            u = UOF[uc] + ul
            for g in range(6):
                W1[g * T:(g + 1) * T, col:col + T] = np.eye(T, dtype=np.float32) * w_vh[g, u]
            for c in range(2):
                W1[96 + c * T:96 + (c + 1) * T, col:col + T] = \
                    np.eye(T, dtype=np.float32) * w_vp[c, u]
            W1[96 + 2 * T, col:col + T] = beta[u]
            col += T

    # pe depthwise band with ones-passthrough row: [3, 29, 29]
    # rows/cols 0:28 = (c,t); row/col 28 = ones lane (identity at ds=1)
    Bpe = np.zeros((3, 2 * T + 1, 2 * T + 1), np.float32)
    for ds in range(3):
        for c in range(2):
            Bpe[ds, c * T:(c + 1) * T, c * T:(c + 1) * T] = _tband(w_hp[ds, :, c])
    Bpe[1, 2 * T, 2 * T] = 1.0

    # conv2 fused band (dw_oh folded with pw_ov): [126, 12*28]
    # col block (uc*3+ds): [uch*14, 28]; rows (u_local,t), cols (k,t')
    # Last uc block is 97 rows: 70 data + 26 zero pad + ones row at 96
    # (xs[3] row 96 is memset to 1.0) which carries beta2 into the psum.
    beta2 = (b_ov + w_ov.T @ b_oh).astype(np.float32)  # [2]
    B2 = np.zeros((126, 12 * (K1 * T)), np.float32)
    for uc in range(4):
        for ds in range(3):
            blk = np.zeros((UCH[uc] * T, K1 * T), np.float32)
            for ul in range(UCH[uc]):
                u = UOF[uc] + ul
                band = _tband(w_oh[ds, :, u])          # [T, T]
                for k in range(K1):
                    blk[ul * T:(ul + 1) * T, k * T:(k + 1) * T] = band * w_ov[u, k]
            c0 = (uc * 3 + ds) * (K1 * T)
            B2[:UCH[uc] * T, c0:c0 + K1 * T] = blk
    for k in range(K1):
        B2[96, (3 * 3 + 1) * (K1 * T) + k * T:(3 * 3 + 1) * (K1 * T) + (k + 1) * T] = beta2[k]

    return {
        "wI": np.eye(128, dtype=np.float32),
        "wIb": np.eye(128, dtype=np.float32).astype(NPBF),
        "wB1": B1.astype(NPBF), "wW1": W1.astype(NPBF),
        "wBpe": Bpe.astype(NPBF), "wB2": B2.astype(NPBF),
    }


def _trace_kernel(nc):
    h1 = nc.dram_tensor("h1", [BPC, S, T, A, 3], F32, kind="ExternalInput").ap()
    h2 = nc.dram_tensor("h2", [BPC, S, T, A, 3], F32, kind="ExternalInput").ap()
    pe = nc.dram_tensor("pe", [BPC, S, T, PEK0], F32, kind="ExternalInput").ap()
    wI = nc.dram_tensor("wI", [128, 128], F32, kind="ExternalInput").ap()
    wIb = nc.dram_tensor("wIb", [128, 128], BF16, kind="ExternalInput").ap()
    wB1 = nc.dram_tensor("wB1", [3, 84, 84], BF16, kind="ExternalInput").ap()
    wW1 = nc.dram_tensor("wW1", [125, 448], BF16, kind="ExternalInput").ap()
    wBpe = nc.dram_tensor("wBpe", [3, 29, 29], BF16, kind="ExternalInput").ap()
    wB2 = nc.dram_tensor("wB2", [126, 336], BF16, kind="ExternalInput").ap()
    y = nc.dram_tensor("y", [BPC, S, T, A, K1], F32, kind="ExternalOutput").ap()

    RELU = mybir.ActivationFunctionType.Relu

    with tile.TileContext(nc) as tc, ExitStack() as ctx:
        wp = ctx.enter_context(tc.tile_pool(name="w", bufs=1))
        hstage_p = ctx.enter_context(tc.tile_pool(name="hstage", bufs=2))
        pestage_p = ctx.enter_context(tc.tile_pool(name="pestage", bufs=2))
        hcp_p = ctx.enter_context(tc.tile_pool(name="hcp", bufs=2))
        pcp_p = ctx.enter_context(tc.tile_pool(name="pcp", bufs=2))
        ht_p = ctx.enter_context(tc.tile_pool(name="ht", bufs=2))
        stk_p = ctx.enter_context(tc.tile_pool(name="stk", bufs=2))
        xs_p = ctx.enter_context(tc.tile_pool(name="xs", bufs=2))
        yt_p = ctx.enter_context(tc.tile_pool(name="ysb", bufs=2))
        stout_p = ctx.enter_context(tc.tile_pool(name="stout", bufs=2))
        psA = ctx.enter_context(tc.tile_pool(name="psA", bufs=2, space="PSUM"))
        psB = ctx.enter_context(tc.tile_pool(name="psB", bufs=4, space="PSUM"))
        psC = ctx.enter_context(tc.tile_pool(name="psC", bufs=2, space="PSUM"))

        ident = wp.tile([128, 128], F32)
        nc.sync.dma_start(ident[:], wI)
        identb = wp.tile([128, 128], BF16)
        nc.sync.dma_start(identb[:], wIb)
        b1 = wp.tile([84, 3, 84], BF16)
        nc.sync.dma_start(b1[:], wB1.rearrange("d k m -> k d m"))
        w1 = wp.tile([125, 448], BF16)
        nc.sync.dma_start(w1[:], wW1)
        bpe = wp.tile([29, 3, 29], BF16)
        nc.sync.dma_start(bpe[:], wBpe.rearrange("d k m -> k d m"))
        b2t = wp.tile([126, 336], BF16)
        nc.sync.dma_start(b2t[:], wB2)
        for b in range(BPC):
            hs = hstage_p.tile([128, 2, NST, T, A, 3], F32, tag="hs")
            for n in range(NST):
                nc.sync.dma_start(hs[:, 0, n], h2[b, n * 128:(n + 1) * 128])
                nc.sync.dma_start(hs[:, 1, n], h1[b, n * 128:(n + 1) * 128])
            ps = pestage_p.tile([128, NST, T, PEK0], F32, tag="ps")
            nc.sync.dma_start(ps[:], pe[b].rearrange("(n p) t c -> p n t c", p=128))

            # ---- per-antenna images (software-pipelined ingest) ----
            def ingest(aa, chunked=False):
                """Stage image aa: strided cast copy, PE transposes to
                channel-major, copy out of PSUM with s-halo. Returns hT.
                When chunked (b=0 first antenna: input DMA still in flight)
                the copies+transposes go per s-tile so the PE can start as
                soon as the first DMA chunks land."""
                hcp = hcp_p.tile([128, NST, 2, 3, T], BF16, tag="hcp",
                                 name=f"hcp{aa}")
                tp = psA.tile([84, S], BF16, tag="s", name=f"tp{aa}")
                if chunked:
                    for j in range(NST):
                        nc.gpsimd.tensor_copy(
                            hcp[:, j, 0], hs[:, 0, j, :, aa, :].rearrange("p t c -> p c t"))
                        nc.scalar.copy(
                            hcp[:, j, 1], hs[:, 1, j, :, aa, :].rearrange("p t c -> p c t"))
                        nc.tensor.transpose(tp[:, j * 128:(j + 1) * 128],
                                            hcp[:, j], identb[:])
                else:
                    nc.gpsimd.tensor_copy(
                        hcp[:, :, 0], hs[:, 0, :, :, aa, :].rearrange("p n t c -> p n c t"))
                    nc.gpsimd.tensor_copy(
                        hcp[:, :, 1], hs[:, 1, :, :, aa, :].rearrange("p n t c -> p n c t"))
                    for j in range(NST):
                        nc.tensor.transpose(tp[:, j * 128:(j + 1) * 128],
                                            hcp[:, j], identb[:])
                hT = ht_p.tile([84, SP], BF16, tag="hT", name=f"hT{aa}")
                if aa < 2:
                    nc.gpsimd.memset(hT[:, 0:1], 0.0)
                    nc.gpsimd.memset(hT[:, SP - 1:SP], 0.0)
                nc.vector.tensor_copy(hT[:, 1:385], tp[:, 0:384])
                nc.scalar.copy(hT[:, 385:769], tp[:, 384:768])
                return hT

            hT_next = ingest(0, chunked=(b == 0))
            # ---- pe branch (per b, shared by all 16 antennas) ----
            # pcp carries a ones lane (col 28) that rides through the
            # transpose + dw band so the stacked pw tile gets its bias row.
            pcp = pcp_p.tile([128, NST, PEK0 * T + 1], BF16, tag="pcp")
            nc.vector.tensor_copy(
                pcp[:, :, 0:28].rearrange("p n (c t) -> p n c t", c=PEK0),
                ps[:].rearrange("p n t c -> p n c t"))
            nc.vector.memset(pcp[:, :, 28:29], 1.0)
            tpe = psA.tile([29, S], BF16, tag="s")
            for j in range(NST):
                nc.tensor.transpose(tpe[0:29, j * 128:(j + 1) * 128],
                                    pcp[:, j], identb[:])
            peT = ht_p.tile([29, SP], BF16, tag="peT")
            nc.vector.memset(peT[:, 0:1], 0.0)
            nc.gpsimd.memset(peT[:, SP - 1:SP], 0.0)
            nc.vector.tensor_copy(peT[0:29, 1:385], tpe[0:29, 0:384])
            nc.scalar.copy(peT[0:29, 385:769], tpe[0:29, 384:768])

            stk0 = stk_p.tile([125, S], BF16, tag="stk0")
            stk1 = stk_p.tile([125, S], BF16, tag="stk1")
            nc.vector.memset(stk0[64:96, :], 0.0)
            nc.gpsimd.memset(stk1[64:96, :], 0.0)
            for s0, sn in CH1:
                dqp = psA.tile([29, sn], F32, tag="s")
                for ds in range(3):
                    nc.tensor.matmul(dqp[0:29, :sn], bpe[:, ds, :],
                                     peT[:, ds + s0: ds + s0 + sn],
                                     start=(ds == 0), stop=(ds == 2))
                nc.scalar.copy(stk0[96:125, s0:s0 + sn], dqp[0:29, :sn])
                nc.vector.tensor_copy(stk1[96:125, s0:s0 + sn], dqp[0:29, :sn])

            stout = stout_p.tile([128, NST, T, A, K1], F32, tag="so")

            for a in range(A):
                stk = stk0 if a % 2 == 0 else stk1
                hT = hT_next

                # dw conv1 -> stack rows 0:84
                for ci, (s0, sn) in enumerate(CH1):
                    dq = psA.tile([84, sn], F32, tag="s")
                    for ds in range(3):
                        nc.tensor.matmul(dq[:, :sn], b1[:, ds, :],
                                         hT[:, ds + s0: ds + s0 + sn],
                                         start=(ds == 0), stop=(ds == 2))
                    if ci == 0:
                        nc.scalar.copy(stk[0:84, s0:s0 + sn], dq[:, :sn])
                    else:
                        nc.vector.tensor_copy(stk[0:84, s0:s0 + sn], dq[:, :sn])

                # pw conv1 (+pe+bias via stacked rhs), relu -> xs
                # xs[3] gets 27 extra rows: zeros at 70:96, ones at 96 —
                # the conv2 lhsT ones-row picks beta2 up from there.
                xts = []
                for uc in range(4):
                    m = 97 if uc == 3 else UCH[uc] * T
                    xt = xs_p.tile([m, SP], BF16, tag=f"x{uc}")
                    xts.append(xt)
                    if a < 2:
                        eng = nc.vector if uc % 2 == 0 else nc.gpsimd
                        eng.memset(xt[:, 0:1], 0.0)
                        eng.memset(xt[:, SP - 1:SP], 0.0)
                        if uc == 3:
                            nc.gpsimd.memset(xt[64:96, :], 0.0)
                            nc.gpsimd.memset(xt[96:97, :], 1.0)
                for s0, sn in CH1:
                    for uc in range(4):
                        m = UCH[uc] * T
                        c0 = UOF[uc] * T
                        px = psB.tile([126, sn], F32, tag="x")
                        nc.tensor.matmul(px[0:m, :sn], w1[:, c0:c0 + m],
                                         stk[:, s0:s0 + sn], start=True, stop=True)
                        dst = xts[uc][0:m, 1 + s0:1 + s0 + sn]
                        if uc == 0 or uc == 2:
                            nc.scalar.activation(dst, px[0:m, :sn], RELU)
                        else:
                            nc.vector.tensor_scalar_max(dst, px[0:m, :sn], 0.0)

                # ingest the next image while this one's relus drain
                if a + 1 < A:
                    hT_next = ingest(a + 1)

                # conv2 (fused dw+pw banded lhsT, beta2 via xs[3] ones-row)
                yt = yt_p.tile([28, S], F32, tag="yt")
                for ci, (s0, sn) in enumerate(CH2):
                    yq = psC.tile([28, sn], F32, tag="y")
                    kk = 0
                    for uc in range(4):
                        m = 97 if uc == 3 else UCH[uc] * T
                        for ds in range(3):
                            c0 = (uc * 3 + ds) * 28
                            nc.tensor.matmul(yq[:, :sn], b2t[0:m, c0:c0 + 28],
                                             xts[uc][0:m, ds + s0: ds + s0 + sn],
                                             start=(kk == 0), stop=(kk == 11))
                            kk += 1
                    if ci == 0:
                        nc.scalar.copy(yt[:, s0:s0 + sn], yq[:, :sn])
                    else:
                        nc.vector.tensor_copy(yt[:, s0:s0 + sn], yq[:, :sn])

                # egress: transpose back to pixel-major, scatter into stout
                tp2 = psC.tile([128, 168], F32, tag="y")
                for g in range(2):
                    for j in range(3):
                        st = g * 3 + j
                        nc.tensor.transpose(tp2[:, st * 28:(st + 1) * 28],
                                            yt[:, st * 128:(st + 1) * 128],
                                            ident[0:28, 0:28])
                    src = tp2[:, g * 84:(g + 1) * 84].rearrange(
                        "p (n k t) -> p n t k", n=3, k=K1)
                    nc.scalar.copy(stout[:, g * 3:g * 3 + 3, :, a, :], src)

            nc.sync.dma_start(y[b].rearrange("(n p) t a k -> p n t a k", p=128),
                              stout[:])
    nc.compile()
    return nc


_CACHED_NC = None


def get_nc():
    global _CACHED_NC
    if _CACHED_NC is None:
        _CACHED_NC = _trace_kernel(
            bacc.Bacc("TRN2", target_bir_lowering=False, debug=False))
    return _CACHED_NC


def make_in_maps(inputs):
    consts = build_consts(
        inputs["w_hh"], inputs["b_hh"], inputs["w_vh"], inputs["b_vh"],
        inputs["w_hp"], inputs["b_hp"], inputs["w_vp"], inputs["b_vp"],
        inputs["w_oh"], inputs["b_oh"], inputs["w_ov"], inputs["b_ov"])
    in_maps = []
    for i in range(NCORES):
        sl = slice(i * BPC, (i + 1) * BPC)
        m = {
            "h1": np.ascontiguousarray(inputs["h1"][sl], np.float32),
            "h2": np.ascontiguousarray(inputs["h2"][sl], np.float32),
            "pe": np.ascontiguousarray(inputs["pe"][sl], np.float32),
        }
        m.update(consts)
        in_maps.append(m)
    return in_maps


def kernel(**inputs):
    nc = get_nc()
    in_maps = make_in_maps(inputs)
    res = run_bass_kernel_spmd(nc, in_maps, list(range(NCORES)))
    return np.concatenate([r["y"] for r in res.results], axis=0)


# revision 19
# speedup vs baseline: 1.1610x; 1.0131x over previous
"""CGNN message-passing kernel for 8 trn2 NeuronCores.

Algorithm (per image (b,a), image = [S=768, T=14] grid):
  x = pw_vh(dw_hh(concat(h2,h1))) + pw_vp(dw_hp(pe)) + beta   (conv1 + pe branch)
  x = relu(x)
  y = pw_ov(dw_oh(x)) + beta2                                 (conv2)

Layout strategy: channel-major SBUF tiles [(chan,t) partitions, s free].
Depthwise 3x3 convs become 3 accumulating matmuls (one per s-shift ds) with
host-precomputed banded lhsT matrices that encode the t-direction taps
(T=14 blocks on the partition axis). conv2's depthwise+pointwise are fused
into a single banded lhsT per (u-chunk, ds). The pe branch and all biases
are folded into the conv1 pointwise contraction via a stacked rhs tile
[hd(84); pedw(28); ones(1)]. All matmuls run in bf16 (4x PE throughput);
PSUM accumulation stays fp32. Ingest/egress between DRAM pixel-major
layout and channel-major tiles uses PE transposes.

Sharding: data-parallel over batch B=16 -> 2 batches per core.
"""

import numpy as np
import ml_dtypes
from contextlib import ExitStack

import concourse.bass as bass
import concourse.bacc as bacc
import concourse.tile as tile
from concourse import mybir
from concourse.bass_utils import run_bass_kernel_spmd

F32 = mybir.dt.float32
BF16 = mybir.dt.bfloat16
NPBF = ml_dtypes.bfloat16
B, S, T, A = 16, 768, 14, 16
HK0, PEK0, U, K1 = 6, 2, 32, 2
NCORES = 8
BPC = B // NCORES          # batches per core
NST = S // 128             # 6 s-tiles of 128
SP = S + 2                 # s-padded width (zero col at 0 and S+1)
UCH = [9, 9, 9, 5]         # u-chunk sizes (32 = 9+9+9+5)
UOF = [0, 9, 18, 27]
# conv1 (dw + pw) s-chunks: chunk0 extends 2 cols so conv2 chunk0 only
# needs chunk0's relu output (incl. the s=384,385 halo cols).
CH1 = [(0, 386), (386, 382)]
# conv2 s-chunks
CH2 = [(0, 384), (384, 384)]


def _tband(w_t, n_t=T):
    """[n_t, n_t] band matrix M[t, t'] = w_t[t - t' + 1] (3-tap, SAME pad)."""
    m = np.zeros((n_t, n_t), np.float32)
    for t in range(n_t):
        for tp in range(n_t):
            dt = t - tp + 1
            if 0 <= dt <= 2:
                m[t, tp] = w_t[dt]
    return m


def build_consts(w_hh, b_hh, w_vh, b_vh, w_hp, b_hp, w_vp, b_vp,
                 w_oh, b_oh, w_ov, b_ov):
    """Host-side precompute of all lhsT matrices. Returns dict name->array."""
    w_hh = w_hh[:, :, 0, :]   # [3,3,6]
    w_hp = w_hp[:, :, 0, :]   # [3,3,2]
    w_oh = w_oh[:, :, 0, :]   # [3,3,32]

    # conv1 depthwise band: [3, 84, 84], rows/cols = g*14+t, g = concat chan
    B1 = np.zeros((3, 6 * T, 6 * T), np.float32)
    for ds in range(3):
        for g in range(6):
            B1[ds, g * T:(g + 1) * T, g * T:(g + 1) * T] = _tband(w_hh[ds, :, g])

    # conv1 pointwise with stacked pe rows + bias row: [125, 448]
    # rows 0:84 = hd rows (g,t'); 84:96 zero pad (32-aligned pe block);
    # 96:124 = pedw rows (c,t'); 124 = ones row (carries beta)
    ncol = sum(u * T for u in UCH)
    beta = (b_vh + w_vh.T @ b_hh + b_vp + w_vp.T @ b_hp).astype(np.float32)  # [32]
    W1 = np.zeros((96 + 2 * T + 1, ncol), np.float32)
    col = 0
    for uc in range(4):
        for ul in range(UCH[uc]):
            u = UOF[uc] + ul
            for g in range(6):
                W1[g * T:(g + 1) * T, col:col + T] = np.eye(T, dtype=np.float32) * w_vh[g, u]
            for c in range(2):
                W1[96 + c * T:96 + (c + 1) * T, col:col + T] = \
                    np.eye(T, dtype=np.float32) * w_vp[c, u]
            W1[96 + 2 * T, col:col + T] = beta[u]
            col += T

    # pe depthwise band with ones-passthrough row: [3, 29, 29]
    # rows/cols 0:28 = (c,t); row/col 28 = ones lane (identity at ds=1)
    Bpe = np.zeros((3, 2 * T + 1, 2 * T + 1), np.float32)
    for ds in range(3):
        for c in range(2):
            Bpe[ds, c * T:(c + 1) * T, c * T:(c + 1) * T] = _tband(w_hp[ds, :, c])
    Bpe[1, 2 * T, 2 * T] = 1.0

    # conv2 fused band (dw_oh folded with pw_ov): [126, 12*28]
    # col block (uc*3+ds): [uch*14, 28]; rows (u_local,t), cols (k,t')
    # Last uc block is 97 rows: 70 data + 26 zero pad + ones row at 96
    # (xs[3] row 96 is memset to 1.0) which carries beta2 into the psum.
    beta2 = (b_ov + w_ov.T @ b_oh).astype(np.float32)  # [2]
    B2 = np.zeros((126, 12 * (K1 * T)), np.float32)
    for uc in range(4):
        for ds in range(3):
            blk = np.zeros((UCH[uc] * T, K1 * T), np.float32)
            for ul in range(UCH[uc]):
                u = UOF[uc] + ul
                band = _tband(w_oh[ds, :, u])          # [T, T]
                for k in range(K1):
                    blk[ul * T:(ul + 1) * T, k * T:(k + 1) * T] = band * w_ov[u, k]
            c0 = (uc * 3 + ds) * (K1 * T)
            B2[:UCH[uc] * T, c0:c0 + K1 * T] = blk
    for k in range(K1):
        B2[96, (3 * 3 + 1) * (K1 * T) + k * T:(3 * 3 + 1) * (K1 * T) + (k + 1) * T] = beta2[k]

    return {
        "wI": np.eye(128, dtype=np.float32),
        "wIb": np.eye(128, dtype=np.float32).astype(NPBF),
        "wB1": B1.astype(NPBF), "wW1": W1.astype(NPBF),
        "wBpe": Bpe.astype(NPBF), "wB2": B2.astype(NPBF),
    }


def _trace_kernel(nc):
    h1 = nc.dram_tensor("h1", [BPC, S, T, A, 3], BF16, kind="ExternalInput").ap()
    h2 = nc.dram_tensor("h2", [BPC, S, T, A, 3], BF16, kind="ExternalInput").ap()
    pe = nc.dram_tensor("pe", [BPC, S, T, PEK0], BF16, kind="ExternalInput").ap()
    wI = nc.dram_tensor("wI", [128, 128], F32, kind="ExternalInput").ap()
    wIb = nc.dram_tensor("wIb", [128, 128], BF16, kind="ExternalInput").ap()
    wB1 = nc.dram_tensor("wB1", [3, 84, 84], BF16, kind="ExternalInput").ap()
    wW1 = nc.dram_tensor("wW1", [125, 448], BF16, kind="ExternalInput").ap()
    wBpe = nc.dram_tensor("wBpe", [3, 29, 29], BF16, kind="ExternalInput").ap()
    wB2 = nc.dram_tensor("wB2", [126, 336], BF16, kind="ExternalInput").ap()
    y = nc.dram_tensor("y", [BPC, S, T, A, K1], BF16, kind="ExternalOutput").ap()

    RELU = mybir.ActivationFunctionType.Relu

    with tile.TileContext(nc) as tc, ExitStack() as ctx:
        wp = ctx.enter_context(tc.tile_pool(name="w", bufs=1))
        hstage_p = ctx.enter_context(tc.tile_pool(name="hstage", bufs=2))
        pestage_p = ctx.enter_context(tc.tile_pool(name="pestage", bufs=2))
        hcp_p = ctx.enter_context(tc.tile_pool(name="hcp", bufs=2))
        pcp_p = ctx.enter_context(tc.tile_pool(name="pcp", bufs=2))
        ht_p = ctx.enter_context(tc.tile_pool(name="ht", bufs=2))
        stk_p = ctx.enter_context(tc.tile_pool(name="stk", bufs=2))
        xs_p = ctx.enter_context(tc.tile_pool(name="xs", bufs=2))
        yt_p = ctx.enter_context(tc.tile_pool(name="ysb", bufs=2))
        stout_p = ctx.enter_context(tc.tile_pool(name="stout", bufs=2))
        psA = ctx.enter_context(tc.tile_pool(name="psA", bufs=2, space="PSUM"))
        psB = ctx.enter_context(tc.tile_pool(name="psB", bufs=4, space="PSUM"))
        psC = ctx.enter_context(tc.tile_pool(name="psC", bufs=2, space="PSUM"))

        ident = wp.tile([128, 128], F32)
        nc.sync.dma_start(ident[:], wI)
        identb = wp.tile([128, 128], BF16)
        nc.sync.dma_start(identb[:], wIb)
        b1 = wp.tile([84, 3, 84], BF16)
        nc.sync.dma_start(b1[:], wB1.rearrange("d k m -> k d m"))
        w1 = wp.tile([125, 448], BF16)
        nc.sync.dma_start(w1[:], wW1)
        bpe = wp.tile([29, 3, 29], BF16)
        nc.sync.dma_start(bpe[:], wBpe.rearrange("d k m -> k d m"))
        b2t = wp.tile([126, 336], BF16)
        nc.sync.dma_start(b2t[:], wB2)
        for b in range(BPC):
            hs = hstage_p.tile([128, 2, NST, T, A, 3], BF16, tag="hs")
            for n in range(NST):
                nc.sync.dma_start(hs[:, 0, n], h2[b, n * 128:(n + 1) * 128])
                nc.sync.dma_start(hs[:, 1, n], h1[b, n * 128:(n + 1) * 128])
            ps = pestage_p.tile([128, NST, T, PEK0], BF16, tag="ps")
            nc.sync.dma_start(ps[:], pe[b].rearrange("(n p) t c -> p n t c", p=128))

            # ---- per-antenna images (software-pipelined ingest) ----
            def ingest(aa, chunked=False):
                """Stage image aa: strided cast copy, PE transposes to
                channel-major, copy out of PSUM with s-halo. Returns hT.
                When chunked (b=0 first antenna: input DMA still in flight)
                the copies+transposes go per s-tile so the PE can start as
                soon as the first DMA chunks land."""
                hcp = hcp_p.tile([128, NST, 2, 3, T], BF16, tag="hcp",
                                 name=f"hcp{aa}")
                tp = psA.tile([84, S], BF16, tag="s", name=f"tp{aa}")
                if chunked:
                    for j in range(NST):
                        nc.gpsimd.tensor_copy(
                            hcp[:, j, 0], hs[:, 0, j, :, aa, :].rearrange("p t c -> p c t"))
                        nc.scalar.copy(
                            hcp[:, j, 1], hs[:, 1, j, :, aa, :].rearrange("p t c -> p c t"))
                        nc.tensor.transpose(tp[:, j * 128:(j + 1) * 128],
                                            hcp[:, j], identb[:])
                else:
                    nc.gpsimd.tensor_copy(
                        hcp[:, :, 0], hs[:, 0, :, :, aa, :].rearrange("p n t c -> p n c t"))
                    nc.gpsimd.tensor_copy(
                        hcp[:, :, 1], hs[:, 1, :, :, aa, :].rearrange("p n t c -> p n c t"))
                    for j in range(NST):
                        nc.tensor.transpose(tp[:, j * 128:(j + 1) * 128],
                                            hcp[:, j], identb[:])
                hT = ht_p.tile([84, SP], BF16, tag="hT", name=f"hT{aa}")
                if aa < 2:
                    nc.gpsimd.memset(hT[:, 0:1], 0.0)
                    nc.gpsimd.memset(hT[:, SP - 1:SP], 0.0)
                nc.vector.tensor_copy(hT[:, 1:385], tp[:, 0:384])
                nc.scalar.copy(hT[:, 385:769], tp[:, 384:768])
                return hT

            hT_next = ingest(0, chunked=(b == 0))
            # ---- pe branch (per b, shared by all 16 antennas) ----
            # pcp carries a ones lane (col 28) that rides through the
            # transpose + dw band so the stacked pw tile gets its bias row.
            pcp = pcp_p.tile([128, NST, PEK0 * T + 1], BF16, tag="pcp")
            nc.vector.tensor_copy(
                pcp[:, :, 0:28].rearrange("p n (c t) -> p n c t", c=PEK0),
                ps[:].rearrange("p n t c -> p n c t"))
            nc.vector.memset(pcp[:, :, 28:29], 1.0)
            tpe = psA.tile([29, S], BF16, tag="s")
            for j in range(NST):
                nc.tensor.transpose(tpe[0:29, j * 128:(j + 1) * 128],
                                    pcp[:, j], identb[:])
            peT = ht_p.tile([29, SP], BF16, tag="peT")
            nc.vector.memset(peT[:, 0:1], 0.0)
            nc.gpsimd.memset(peT[:, SP - 1:SP], 0.0)
            nc.vector.tensor_copy(peT[0:29, 1:385], tpe[0:29, 0:384])
            nc.scalar.copy(peT[0:29, 385:769], tpe[0:29, 384:768])

            stk0 = stk_p.tile([125, S], BF16, tag="stk0")
            stk1 = stk_p.tile([125, S], BF16, tag="stk1")
            nc.vector.memset(stk0[64:96, :], 0.0)
            nc.gpsimd.memset(stk1[64:96, :], 0.0)
            for s0, sn in CH1:
                dqp = psA.tile([29, sn], F32, tag="s")
                for ds in range(3):
                    nc.tensor.matmul(dqp[0:29, :sn], bpe[:, ds, :],
                                     peT[:, ds + s0: ds + s0 + sn],
                                     start=(ds == 0), stop=(ds == 2))
                nc.scalar.copy(stk0[96:125, s0:s0 + sn], dqp[0:29, :sn])
                nc.vector.tensor_copy(stk1[96:125, s0:s0 + sn], dqp[0:29, :sn])

            stout = stout_p.tile([128, NST, T, A, K1], BF16, tag="so")

            for a in range(A):
                stk = stk0 if a % 2 == 0 else stk1
                hT = hT_next

                # dw conv1 -> stack rows 0:84
                for ci, (s0, sn) in enumerate(CH1):
                    dq = psA.tile([84, sn], F32, tag="s")
                    for ds in range(3):
                        nc.tensor.matmul(dq[:, :sn], b1[:, ds, :],
                                         hT[:, ds + s0: ds + s0 + sn],
                                         start=(ds == 0), stop=(ds == 2))
                    if ci == 0:
                        nc.scalar.copy(stk[0:84, s0:s0 + sn], dq[:, :sn])
                    else:
                        nc.vector.tensor_copy(stk[0:84, s0:s0 + sn], dq[:, :sn])

                # pw conv1 (+pe+bias via stacked rhs), relu -> xs
                # xs[3] gets 27 extra rows: zeros at 70:96, ones at 96 —
                # the conv2 lhsT ones-row picks beta2 up from there.
                xts = []
                for uc in range(4):
                    m = 97 if uc == 3 else UCH[uc] * T
                    xt = xs_p.tile([m, SP], BF16, tag=f"x{uc}")
                    xts.append(xt)
                    if a < 2:
                        eng = nc.vector if uc % 2 == 0 else nc.gpsimd
                        eng.memset(xt[:, 0:1], 0.0)
                        eng.memset(xt[:, SP - 1:SP], 0.0)
                        if uc == 3:
                            nc.gpsimd.memset(xt[64:96, :], 0.0)
                            nc.gpsimd.memset(xt[96:97, :], 1.0)
                for s0, sn in CH1:
                    for uc in range(4):
                        m = UCH[uc] * T
                        c0 = UOF[uc] * T
                        px = psB.tile([126, sn], F32, tag="x")
                        nc.tensor.matmul(px[0:m, :sn], w1[:, c0:c0 + m],
                                         stk[:, s0:s0 + sn], start=True, stop=True)
                        dst = xts[uc][0:m, 1 + s0:1 + s0 + sn]
                        if uc == 0 or uc == 2:
                            nc.scalar.activation(dst, px[0:m, :sn], RELU)
                        else:
                            nc.vector.tensor_scalar_max(dst, px[0:m, :sn], 0.0)

                # ingest the next image while this one's relus drain
                if a + 1 < A:
                    hT_next = ingest(a + 1)

                # conv2 (fused dw+pw banded lhsT, beta2 via xs[3] ones-row)
                yt = yt_p.tile([28, S], F32, tag="yt")
                for ci, (s0, sn) in enumerate(CH2):
                    yq = psC.tile([28, sn], F32, tag="y")
                    kk = 0
                    for uc in range(4):
                        m = 97 if uc == 3 else UCH[uc] * T
                        for ds in range(3):
                            c0 = (uc * 3 + ds) * 28
                            nc.tensor.matmul(yq[:, :sn], b2t[0:m, c0:c0 + 28],
                                             xts[uc][0:m, ds + s0: ds + s0 + sn],
                                             start=(kk == 0), stop=(kk == 11))
                            kk += 1
                    if ci == 0:
                        nc.scalar.copy(yt[:, s0:s0 + sn], yq[:, :sn])
                    else:
                        nc.vector.tensor_copy(yt[:, s0:s0 + sn], yq[:, :sn])

                # egress: transpose back to pixel-major, scatter into stout
                tp2 = psC.tile([128, 168], F32, tag="y")
                for g in range(2):
                    for j in range(3):
                        st = g * 3 + j
                        nc.tensor.transpose(tp2[:, st * 28:(st + 1) * 28],
                                            yt[:, st * 128:(st + 1) * 128],
                                            ident[0:28, 0:28])
                    src = tp2[:, g * 84:(g + 1) * 84].rearrange(
                        "p (n k t) -> p n t k", n=3, k=K1)
                    nc.scalar.copy(stout[:, g * 3:g * 3 + 3, :, a, :], src)

            nc.sync.dma_start(y[b].rearrange("(n p) t a k -> p n t a k", p=128),
                              stout[:])
    nc.compile()
    return nc


_CACHED_NC = None


def get_nc():
    global _CACHED_NC
    if _CACHED_NC is None:
        _CACHED_NC = _trace_kernel(
            bacc.Bacc("TRN2", target_bir_lowering=False, debug=False))
    return _CACHED_NC


def make_in_maps(inputs):
    consts = build_consts(
        inputs["w_hh"], inputs["b_hh"], inputs["w_vh"], inputs["b_vh"],
        inputs["w_hp"], inputs["b_hp"], inputs["w_vp"], inputs["b_vp"],
        inputs["w_oh"], inputs["b_oh"], inputs["w_ov"], inputs["b_ov"])
    in_maps = []
    for i in range(NCORES):
        sl = slice(i * BPC, (i + 1) * BPC)
        m = {
            "h1": np.asarray(inputs["h1"][sl]).astype(NPBF),
            "h2": np.asarray(inputs["h2"][sl]).astype(NPBF),
            "pe": np.asarray(inputs["pe"][sl]).astype(NPBF),
        }
        m.update(consts)
        in_maps.append(m)
    return in_maps


def kernel(**inputs):
    nc = get_nc()
    in_maps = make_in_maps(inputs)
    res = run_bass_kernel_spmd(nc, in_maps, list(range(NCORES)))
    return np.concatenate([r["y"] for r in res.results], axis=0).astype(np.float32)
